# revision 27
# baseline (speedup 1.0000x reference)
"""BERT-CRF NER Viterbi decode kernel for Trainium2 (8 NeuronCores).

Data-parallel over batch (8 rows/core), raw Bass.  Device does the heavy
work — streaming hidden_states and the sequential Viterbi forward scan:

  - host: shard + transpose hidden_states, split into bf16 hi/lo pairs
    (exact to ~3e-5), permute the t-axis into 2 step-groups so the scan
    starts while the second half is still streaming in.
  - device (per core):
      feats = W.T @ hsT per (batch-row pair, group): bf16 hi/lo 3-part
      matmuls (hi@Wh + hi@Wl + lo@Wh, fp32 PSUM) into one PSUM region per
      group; spread DMAs issued from 3 engine queues.
      CHUNKED Viterbi forward scan: T=512 -> 16 chunks x 32 steps, chunk c
      covers t in [32c+1, 32c+32]; partition row r = 16b + c.  Each chunk
      scans a max-plus MATRIX state S[f0,to] (7x7, f0 = unknown boundary
      label): 1 seed + 31 sequential (TT + reduce-max) ops on [128,343]
      instead of 510 dependent steps on [8,56].  Chunk 0's START seed is
      baked into per-row values of the trT constant.  Group 0's 16 steps run
      under group 1's DMA; prefix states stream out per group.
  - host: chunk-boundary combine (15 tiny max-plus matvecs), delta finalize
    (elementwise max over f0 — same fp32 ops the DVE would do, bit-exact),
    then the backtrace pointer-chase the baseline already did host-side.
"""

import numpy as np
from contextlib import ExitStack

import concourse.bass as bass
from concourse import mybir
from concourse.bass_utils import run_bass_kernel_spmd

B, T, H = 64, 512, 768
NC = 8              # cores
BL = B // NC        # batch rows per core = 8
KC = H // 128       # 6 contraction chunks
CH = 16             # time chunks
CL = T // CH        # 32 steps per chunk
NG = 2              # step groups (phase-1 streaming)
GL = CL // NG       # 16 steps per group
GT = T // NG        # 256 t per group per batch row
START = 7
STOP = 8

F32 = mybir.dt.float32
BF16 = mybir.dt.bfloat16
ADD = mybir.AluOpType.add
MAX = mybir.AluOpType.max
AXX = mybir.AxisListType.X

LAB = [0, 1, 2, 3, 4, 5, 6, 8]   # compact label order (START dropped)
LC = 8                           # compact labels incl STOP (feats)
SC = 7                           # scan state width: 'to'/'from' in 0..6
SQ = SC * SC                     # 49


def build_program():
    nc = bass.Bass("TRN2", target_bir_lowering=False,
                   detect_race_conditions=False)

    hsTh_d = nc.dram_tensor("hsTh", [BL, H, T], BF16, kind="ExternalInput")
    hsTl_d = nc.dram_tensor("hsTl", [BL, H, T], BF16, kind="ExternalInput")
    wkh_d = nc.dram_tensor("wkh", [128, KC * LC], BF16, kind="ExternalInput")
    wkl_d = nc.dram_tensor("wkl", [128, KC * LC], BF16, kind="ExternalInput")
    trR_d = nc.dram_tensor("trR", [128, SQ], F32, kind="ExternalInput")
    trT_d = nc.dram_tensor("trT", [128, SQ], F32, kind="ExternalInput")
    sall_d = nc.dram_tensor("sall", [128, CL * SQ], F32,
                            kind="ExternalOutput")
    feats_d = nc.dram_tensor("feats", [128, LC * CL], F32,
                             kind="ExternalOutput")

    with ExitStack() as ctx:
        def sb(name, shape, dt=F32):
            return ctx.enter_context(nc.sbuf_tensor(name, shape, dt))
        wkh = sb("wkh_sb", [128, KC * LC], BF16)
        wkl = sb("wkl_sb", [128, KC * LC], BF16)
        trR = sb("trR_sb", [128, SQ])       # (to,f): trans[to,f]+b[to]
        trT = sb("trT_sb", [128, SQ])       # (f0,to); chunk-0 rows: START col
        hth = sb("hth_all", [128, BL * KC * T], BF16)   # (b, kc, u)
        htl = sb("htl_all", [128, BL * KC * T], BF16)
        feats_sp = sb("feats_sp", [128, LC * CL])   # [(b,c), (l,j)] l-major
        Mt = sb("Mt", [128, CL * SQ])       # [(b,c), (j,to,f)]
        Sall = sb("Sall", [128, CL * SQ])   # [(b,c), (j,f0,to)]
        scsc = sb("scsc", [128, SC * SQ])   # scan scratch (f0,to,f)
        stage = [sb(f"stage{g}", [LC, BL * GT]) for g in range(NG)]
        # stage[g]: [to, (b, c, jj)] — group g's feats, contiguous per row
        psum = [ctx.enter_context(nc.psum_tensor(f"psumg{g}", [LC, BL * GT],
                                                 F32)) for g in range(NG)]

        in_sem = ctx.enter_context(nc.semaphore("in_sem"))
        hs_sems = [ctx.enter_context(nc.semaphore(f"hs_sem{i}"))
                   for i in range(BL)]
        pe_sem = ctx.enter_context(nc.semaphore("pe_sem"))
        cp_sem = ctx.enter_context(nc.semaphore("cp_sem"))
        sp_sem = ctx.enter_context(nc.semaphore("sp_sem"))
        dv_sem = ctx.enter_context(nc.semaphore("dv_sem"))
        out_sem = ctx.enter_context(nc.semaphore("out_sem"))
        block = ctx.enter_context(nc.Block())

        def spread(eng, l, g):
            # stage[g][l, b*256 + c*16 + jj] -> feats_sp[(b,c), l*32 + g*16
            #   + jj];  u-order already encodes t = 32c + 16g + jj + 1
            src = stage[g][l:l + 1, :]
            dst = feats_sp[:, l * CL + g * GL:l * CL + (g + 1) * GL]
            eng.dma_start(dst, src).then_inc(sp_sem, 16)

        @block.gpsimd
        def _(g):
            g.dma_start(wkh[:, :], wkh_d[:, :]).then_inc(in_sem, 16)
            g.dma_start(wkl[:, :], wkl_d[:, :]).then_inc(in_sem, 16)
            g.dma_start(trR[:, :], trR_d[:, :]).then_inc(in_sem, 16)
            g.dma_start(trT[:, :], trT_d[:, :]).then_inc(in_sem, 16)
            for gg in range(NG):
                g.wait_ge(cp_sem, BL * (gg + 1))
                for l in range(0, 3):
                    spread(g, l, gg)

        def load_bg(sync, b, g):
            for src_d, dst_t in ((hsTh_d, hth), (hsTl_d, htl)):
                src = (src_d[b, :, :].rearrange("(kc p) u -> p kc u", p=128)
                       [:, :, g * GT:(g + 1) * GT])
                dst = (dst_t[:, :]
                       .rearrange("p (b kc u) -> p b kc u", kc=KC, u=T)
                       [:, b, :, g * GT:(g + 1) * GT])
                sync.dma_start(dst, src).then_inc(hs_sems[b], 16)

        @block.sync
        def _(sync):
            for b in range(BL):
                load_bg(sync, b, 0)
            for b in range(BL):
                load_bg(sync, b, 1)
            for gg in range(NG):
                sync.wait_ge(cp_sem, BL * (gg + 1))
                for l in range(6, 8):
                    spread(sync, l, gg)
            # export feats (host needs the t=511 row; cheap to send all)
            sync.wait_ge(sp_sem, 16 * LC * NG)
            sync.dma_start(feats_d[:, :], feats_sp[:, :]).then_inc(out_sem, 16)
            # export scan prefix states per group (host does phases 2+3)
            sync.wait_ge(dv_sem, 1)
            sync.dma_start(sall_d[:, 0:GL * SQ],
                           Sall[:, 0:GL * SQ]).then_inc(out_sem, 16)
            sync.wait_ge(dv_sem, 2)
            sync.dma_start(sall_d[:, GL * SQ:CL * SQ],
                           Sall[:, GL * SQ:CL * SQ]).then_inc(out_sem, 16)

        @block.tensor
        def _(te):
            te.wait_ge(in_sem, 64)
            for g in range(NG):
                for q in range(BL // 2):       # batch-row pairs
                    te.wait_ge(hs_sems[2 * q], 32 * (g + 1))
                    te.wait_ge(hs_sems[2 * q + 1], 32 * (g + 1))
                    parts = ((wkh, hth), (wkl, hth), (wkh, htl))
                    for kc in range(KC):
                        for i, (w, x) in enumerate(parts):
                            mv = (x[:, :].rearrange(
                                      "p (b kc u) -> p b kc u", kc=KC, u=T)
                                  [:, 2 * q:2 * q + 2, kc,
                                   g * GT:(g + 1) * GT])
                            m = te.matmul(
                                psum[g][:, 2 * q * GT:(2 * q + 2) * GT],
                                w[:, kc * LC:(kc + 1) * LC],
                                mv,
                                start=(kc == 0 and i == 0),
                                stop=(kc == KC - 1 and i == len(parts) - 1),
                            )
                            if kc == KC - 1 and i == len(parts) - 1:
                                m.then_inc(pe_sem, 2)

        @block.scalar
        def _(act):
            for g in range(NG):
                for q in range(BL // 2):
                    act.wait_ge(pe_sem, BL * g + 2 * q + 2)
                    act.copy(stage[g][:, 2 * q * GT:(2 * q + 2) * GT],
                             psum[g][:, 2 * q * GT:(2 * q + 2) * GT]
                             ).then_inc(cp_sem, 2)
                for l in range(3, 6):
                    spread(act, l, g)

        @block.vector
        def _(v):
            v.wait_ge(in_sem, 64)

            def mbuild(g):
                # M[j,to,f] = trans[to,f]+b[to]+feat_{32c+1+j}[to]
                m4 = (Mt[:, g * GL * SQ:(g + 1) * GL * SQ]
                      .rearrange("p (j t f) -> p j t f", t=SC, f=SC))
                tr4 = (trR[:, :].rearrange("p (t f) -> p t f", f=SC)
                       .unsqueeze(1).broadcast_to([128, GL, SC, SC]))
                ft4 = (feats_sp[:, :].rearrange("p (l j) -> p j l", l=LC)
                       [:, g * GL:(g + 1) * GL, 0:SC]
                       .unsqueeze(3).broadcast_to([128, GL, SC, SC]))
                v.tensor_tensor(m4, tr4, ft4, op=ADD)

            def step(j):
                # scores[f0,to,f] = M_j[to,f] + S_{j-1}[f0,f]
                in0 = (Mt[:, j * SQ:(j + 1) * SQ]
                       .rearrange("p (t f) -> p t f", f=SC)
                       .unsqueeze(1).broadcast_to([128, SC, SC, SC]))
                in1 = (Sall[:, (j - 1) * SQ:j * SQ]
                       .rearrange("p (f0 t) -> p f0 t", t=SC)
                       .unsqueeze(2).broadcast_to([128, SC, SC, SC]))
                o4 = scsc[:, :].rearrange("p (f0 t f) -> p f0 t f",
                                          t=SC, f=SC)
                v.tensor_tensor(o4, in0, in1, op=ADD)
                v.tensor_reduce(
                    Sall[:, j * SQ:(j + 1) * SQ]
                    .rearrange("p (f0 t) -> p f0 t", t=SC),
                    o4, axis=AXX, op=MAX)
                v.engine_nop()

            v.wait_ge(sp_sem, 16 * LC)          # group-0 feats in place
            mbuild(0)
            # seed slot 0: S_0[f0,to] = trT[f0,to] + feat_{32c+1}[to]
            s0 = Sall[:, 0:SQ].rearrange("p (f t) -> p f t", t=SC)
            tT3 = trT[:, :].rearrange("p (f t) -> p f t", t=SC)
            f0b = (feats_sp[:, :].rearrange("p (l j) -> p j l", l=LC)
                   [:, 0:1, 0:SC].broadcast_to([128, SC, SC]))
            v.tensor_tensor(s0, tT3, f0b, op=ADD)
            v.engine_nop()
            for j in range(1, GL):
                step(j)
            v.engine_nop().then_inc(dv_sem, 1)  # group-0 states exportable
            v.wait_ge(sp_sem, 16 * LC * NG)     # group-1 feats in place
            mbuild(1)
            for j in range(GL, CL):
                step(j)
            v.engine_nop().then_inc(dv_sem, 1)

    return nc


_PROG = None


def _get_prog():
    global _PROG
    if _PROG is None:
        _PROG = build_program()
    return _PROG


def _perm_t():
    """u -> t map: u = g*256 + c*16 + jj holds feat time t = 32c+16g+jj+1."""
    u = np.arange(T)
    g = u // GT
    c = (u % GT) // GL
    jj = u % GL
    return 32 * c + GL * g + jj + 1          # in 1..512; 512 -> zero-fill


def make_in_maps(hidden_states, W, b, transitions):
    hs = np.asarray(hidden_states, np.float32)
    W = np.asarray(W, np.float32)
    bb = np.asarray(b, np.float32)
    trans = np.asarray(transitions, np.float32)

    import ml_dtypes
    BF = ml_dtypes.bfloat16
    Wc = W[:, LAB]                                       # [768, 8]
    wk = np.ascontiguousarray(Wc.reshape(KC, 128, LC).transpose(1, 0, 2)
                              ).reshape(128, KC * LC)
    wkh = wk.astype(BF)
    wkl = (wk - wkh.astype(np.float32)).astype(BF)
    trb = trans[0:SC, 0:SC] + bb[0:SC, None]             # [to, f] (+bias)
    trR = np.ascontiguousarray(
        np.broadcast_to(trb.reshape(1, SQ), (128, SQ)))
    trT = np.ascontiguousarray(
        np.broadcast_to(trb.T.reshape(1, SQ), (128, SQ))).copy()
    # chunk-0 rows (r = 16b): seed from START: trans[to,7]+b[to], any f0
    d7 = trans[0:SC, START] + bb[0:SC]
    trT[0::CH, :] = np.broadcast_to(d7[None, :], (SC, SC)).reshape(SQ)

    t_of_u = _perm_t()
    tclip = np.minimum(t_of_u, T - 1)
    dead = t_of_u >= T
    in_maps = []
    for c in range(NC):
        shard = hs[c * BL:(c + 1) * BL]                  # [8, 512, 768]
        hsT = np.ascontiguousarray(shard.transpose(0, 2, 1))  # [8, 768, 512]
        hsP = hsT[:, :, tclip]
        hsP[:, :, dead] = 0.0
        hsTh = hsP.astype(BF)
        hsTl = (hsP - hsTh.astype(np.float32)).astype(BF)
        in_maps.append({"hsTh": hsTh, "hsTl": hsTl, "wkh": wkh, "wkl": wkl,
                        "trR": trR, "trT": trT})
    return in_maps


def kernel(hidden_states, W, b, transitions):
    in_maps = make_in_maps(hidden_states, W, b, transitions)
    nc = _get_prog()
    res = run_bass_kernel_spmd(nc, in_maps, list(range(NC))).results

    bb = np.asarray(b, np.float32)
    trans = np.asarray(transitions, np.float32)
    lab = np.array(LAB, np.int32)

    # scan prefix states: rows r = 16b + c, slots (j, f0, to)
    sall = np.stack([res[c]["sall"] for c in range(NC)])     # [8,128,1568]
    S = sall.reshape(B, CH, CL, SC, SC)                      # [bt,c,j,f0,to]
    feats = np.stack([res[c]["feats"] for c in range(NC)])   # [8,128,256]
    feats = (feats.reshape(NC, BL, CH, LC, CL)
             .transpose(0, 1, 2, 4, 3).reshape(B, T, LC))    # s <-> t=s+1

    # phase 2 (host): boundary deltas across chunks, same fp32 ops as device
    db = np.zeros((B, CH, SC), np.float32)
    for c in range(CH - 1):
        db[:, c + 1] = (S[:, c, CL - 1] + db[:, c][:, :, None]).max(axis=1)
    # phase 3 (host): delta_t = max_f0(S_t[f0,:] + db_c[f0])
    delta = (S + db[:, :, None, :, None]).max(axis=3)        # [bt,c,j,to]
    dall = np.empty((B, T + 1, SC), np.float32)
    dall[:, 1:] = delta.reshape(B, T, SC)                    # slot s <-> t=s+1

    # final step: candidates over compact labels (0..6, STOP)
    d510 = dall[:, T - 2, :]                                 # [64, 7]
    cand = np.empty((B, LC), np.float32)
    cand[:, 0:SC] = dall[:, T - 1, :]
    cand[:, SC] = (np.max(trans[STOP, 0:SC][None, :] + d510, axis=1)
                   + feats[:, T - 2, SC] + bb[STOP])
    p = lab[np.argmax(cand, axis=1)]                         # [64] labels
    path = np.empty((B, T), np.int32)
    path[:, T - 1] = p
    trf = trans[:, 0:SC]                                     # [9, 7]
    for t in range(T - 1, 1, -1):
        p = np.argmax(trf[p] + dall[:, t - 1, :], axis=1).astype(np.int32)
        path[:, t - 1] = p
    path[:, 0] = START
    return path


# revision 28
# speedup vs baseline: 1.0993x; 1.0993x over previous
"""BERT-CRF NER Viterbi decode kernel for Trainium2 (8 NeuronCores).

Data-parallel over batch (8 rows/core), raw Bass.  Device does the heavy
work — streaming hidden_states and the sequential Viterbi forward scan:

  - host: shard + transpose hidden_states, split into bf16 hi/lo pairs
    (exact to ~3e-5), permute the t-axis into 2 step-groups so the scan
    starts while the second half is still streaming in.
  - device (per core):
      feats = W.T @ hsT per (batch-row pair, group): bf16 hi/lo 3-part
      matmuls (hi@Wh + hi@Wl + lo@Wh, fp32 PSUM) into one PSUM region per
      group; spread DMAs issued from 3 engine queues.
      CHUNKED Viterbi forward scan: T=512 -> 16 chunks x 32 steps, chunk c
      covers t in [32c+1, 32c+32]; partition row r = 16b + c.  Each chunk
      scans a max-plus MATRIX state S[f0,to] (7x7, f0 = unknown boundary
      label): 1 seed + 31 sequential (TT + reduce-max) ops on [128,343]
      instead of 510 dependent steps on [8,56].  Chunk 0's START seed is
      baked into per-row values of the trT constant.  Group 0's 16 steps run
      under group 1's DMA; prefix states stream out per group.
  - host: chunk-boundary combine (15 tiny max-plus matvecs), delta finalize
    (elementwise max over f0 — same fp32 ops the DVE would do, bit-exact),
    then the backtrace pointer-chase the baseline already did host-side.
"""

import numpy as np
from contextlib import ExitStack

import concourse.bass as bass
from concourse import mybir
from concourse.bass_utils import run_bass_kernel_spmd

B, T, H = 64, 512, 768
NC = 8              # cores
BL = B // NC        # batch rows per core = 8
KC = H // 128       # 6 contraction chunks
CH = 16             # time chunks
CL = T // CH        # 32 steps per chunk
NG = 2              # step groups (phase-1 streaming)
GL = CL // NG       # 16 steps per group
GT = T // NG        # 256 t per group per batch row
START = 7
STOP = 8

F32 = mybir.dt.float32
BF16 = mybir.dt.bfloat16
ADD = mybir.AluOpType.add
MAX = mybir.AluOpType.max
AXX = mybir.AxisListType.X

LAB = [0, 1, 2, 3, 4, 5, 6, 8]   # compact label order (START dropped)
LC = 8                           # compact labels incl STOP (feats)
SC = 7                           # scan state width: 'to'/'from' in 0..6
SQ = SC * SC                     # 49


def build_program():
    nc = bass.Bass("TRN2", target_bir_lowering=False,
                   detect_race_conditions=False)

    hsTh_d = nc.dram_tensor("hsTh", [BL, H, T], BF16, kind="ExternalInput")
    hsTl_d = nc.dram_tensor("hsTl", [BL, H, T], BF16, kind="ExternalInput")
    wkh_d = nc.dram_tensor("wkh", [128, KC * LC], BF16, kind="ExternalInput")
    wkl_d = nc.dram_tensor("wkl", [128, KC * LC], BF16, kind="ExternalInput")
    trR_d = nc.dram_tensor("trR", [128, SQ], F32, kind="ExternalInput")
    trT_d = nc.dram_tensor("trT", [128, SQ], F32, kind="ExternalInput")
    sall_d = nc.dram_tensor("sall", [128, CL * SQ], F32,
                            kind="ExternalOutput")
    feats_d = nc.dram_tensor("feats", [128, LC * CL], F32,
                             kind="ExternalOutput")

    with ExitStack() as ctx:
        def sb(name, shape, dt=F32):
            return ctx.enter_context(nc.sbuf_tensor(name, shape, dt))
        wkh = sb("wkh_sb", [128, KC * LC], BF16)
        wkl = sb("wkl_sb", [128, KC * LC], BF16)
        trR = sb("trR_sb", [128, SQ])       # (to,f): trans[to,f]+b[to]
        trT = sb("trT_sb", [128, SQ])       # (f0,to); chunk-0 rows: START col
        hth = sb("hth_all", [128, BL * KC * T], BF16)   # (b, kc, u)
        htl = sb("htl_all", [128, BL * KC * T], BF16)
        feats_sp = sb("feats_sp", [128, LC * CL])   # [(b,c), (l,j)] l-major
        Mt = sb("Mt", [128, CL * SQ])       # [(b,c), (j,to,f)]
        Sall = sb("Sall", [128, CL * SQ])   # [(b,c), (j,f0,to)]
        scsc = sb("scsc", [128, SC * SQ])   # scan scratch (f0,to,f)
        stage = [sb(f"stage{g}", [LC, BL * GT]) for g in range(NG)]
        # stage[g]: [to, (b, c, jj)] — group g's feats, contiguous per row
        psum = [ctx.enter_context(nc.psum_tensor(f"psumg{g}", [LC, BL * GT],
                                                 F32)) for g in range(NG)]

        in_sem = ctx.enter_context(nc.semaphore("in_sem"))
        hs_sems = [ctx.enter_context(nc.semaphore(f"hs_sem{i}"))
                   for i in range(BL)]
        pe_sem = ctx.enter_context(nc.semaphore("pe_sem"))
        cp_sem = ctx.enter_context(nc.semaphore("cp_sem"))
        sp_sem = ctx.enter_context(nc.semaphore("sp_sem"))
        dv_sem = ctx.enter_context(nc.semaphore("dv_sem"))
        out_sem = ctx.enter_context(nc.semaphore("out_sem"))
        block = ctx.enter_context(nc.Block())

        def spread(eng, l, g):
            # stage[g][l, b*256 + c*16 + jj] -> feats_sp[(b,c), l*32 + g*16
            #   + jj];  u-order already encodes t = 32c + 16g + jj + 1
            src = stage[g][l:l + 1, :]
            dst = feats_sp[:, l * CL + g * GL:l * CL + (g + 1) * GL]
            eng.dma_start(dst, src).then_inc(sp_sem, 16)

        @block.gpsimd
        def _(g):
            g.dma_start(wkh[:, :], wkh_d[:, :]).then_inc(in_sem, 16)
            g.dma_start(wkl[:, :], wkl_d[:, :]).then_inc(in_sem, 16)
            g.dma_start(trR[:, :], trR_d[:, :]).then_inc(in_sem, 16)
            g.dma_start(trT[:, :], trT_d[:, :]).then_inc(in_sem, 16)
            for gg in range(NG):
                g.wait_ge(cp_sem, BL * (gg + 1))
                for l in range(0, 4):
                    spread(g, l, gg)

        def load_bg(sync, b, g):
            for src_d, dst_t in ((hsTh_d, hth), (hsTl_d, htl)):
                src = (src_d[b, :, :].rearrange("(kc p) u -> p kc u", p=128)
                       [:, :, g * GT:(g + 1) * GT])
                dst = (dst_t[:, :]
                       .rearrange("p (b kc u) -> p b kc u", kc=KC, u=T)
                       [:, b, :, g * GT:(g + 1) * GT])
                sync.dma_start(dst, src).then_inc(hs_sems[b], 16)

        @block.sync
        def _(sync):
            for b in range(BL):
                load_bg(sync, b, 0)
            for b in range(BL):
                load_bg(sync, b, 1)
            # export feats (host needs the t=511 row; cheap to send all)
            sync.wait_ge(sp_sem, 16 * LC * NG)
            sync.dma_start(feats_d[:, :], feats_sp[:, :]).then_inc(out_sem, 16)
            # export scan prefix states per group (host does phases 2+3)
            sync.wait_ge(dv_sem, 1)
            sync.dma_start(sall_d[:, 0:GL * SQ],
                           Sall[:, 0:GL * SQ]).then_inc(out_sem, 16)
            sync.wait_ge(dv_sem, 2)
            sync.dma_start(sall_d[:, GL * SQ:CL * SQ],
                           Sall[:, GL * SQ:CL * SQ]).then_inc(out_sem, 16)

        @block.tensor
        def _(te):
            te.wait_ge(in_sem, 64)
            for g in range(NG):
                for q in range(BL // 2):       # batch-row pairs
                    te.wait_ge(hs_sems[2 * q], 32 * (g + 1))
                    te.wait_ge(hs_sems[2 * q + 1], 32 * (g + 1))
                    parts = ((wkh, hth), (wkl, hth), (wkh, htl))
                    for kc in range(KC):
                        for i, (w, x) in enumerate(parts):
                            mv = (x[:, :].rearrange(
                                      "p (b kc u) -> p b kc u", kc=KC, u=T)
                                  [:, 2 * q:2 * q + 2, kc,
                                   g * GT:(g + 1) * GT])
                            m = te.matmul(
                                psum[g][:, 2 * q * GT:(2 * q + 2) * GT],
                                w[:, kc * LC:(kc + 1) * LC],
                                mv,
                                start=(kc == 0 and i == 0),
                                stop=(kc == KC - 1 and i == len(parts) - 1),
                            )
                            if kc == KC - 1 and i == len(parts) - 1:
                                m.then_inc(pe_sem, 2)

        @block.scalar
        def _(act):
            for g in range(NG):
                for q in range(BL // 2):
                    act.wait_ge(pe_sem, BL * g + 2 * q + 2)
                    act.copy(stage[g][:, 2 * q * GT:(2 * q + 2) * GT],
                             psum[g][:, 2 * q * GT:(2 * q + 2) * GT]
                             ).then_inc(cp_sem, 2)
                for l in range(4, 8):
                    spread(act, l, g)

        @block.vector
        def _(v):
            v.wait_ge(in_sem, 64)

            def mbuild(g):
                # M[j,to,f] = trans[to,f]+b[to]+feat_{32c+1+j}[to]
                m4 = (Mt[:, g * GL * SQ:(g + 1) * GL * SQ]
                      .rearrange("p (j t f) -> p j t f", t=SC, f=SC))
                tr4 = (trR[:, :].rearrange("p (t f) -> p t f", f=SC)
                       .unsqueeze(1).broadcast_to([128, GL, SC, SC]))
                ft4 = (feats_sp[:, :].rearrange("p (l j) -> p j l", l=LC)
                       [:, g * GL:(g + 1) * GL, 0:SC]
                       .unsqueeze(3).broadcast_to([128, GL, SC, SC]))
                v.tensor_tensor(m4, tr4, ft4, op=ADD)

            def step(j):
                # scores[f0,to,f] = M_j[to,f] + S_{j-1}[f0,f]
                in0 = (Mt[:, j * SQ:(j + 1) * SQ]
                       .rearrange("p (t f) -> p t f", f=SC)
                       .unsqueeze(1).broadcast_to([128, SC, SC, SC]))
                in1 = (Sall[:, (j - 1) * SQ:j * SQ]
                       .rearrange("p (f0 t) -> p f0 t", t=SC)
                       .unsqueeze(2).broadcast_to([128, SC, SC, SC]))
                o4 = scsc[:, :].rearrange("p (f0 t f) -> p f0 t f",
                                          t=SC, f=SC)
                v.tensor_tensor(o4, in0, in1, op=ADD)
                v.tensor_reduce(
                    Sall[:, j * SQ:(j + 1) * SQ]
                    .rearrange("p (f0 t) -> p f0 t", t=SC),
                    o4, axis=AXX, op=MAX)
                v.engine_nop()

            v.wait_ge(sp_sem, 16 * LC)          # group-0 feats in place
            mbuild(0)
            # seed slot 0: S_0[f0,to] = trT[f0,to] + feat_{32c+1}[to]
            s0 = Sall[:, 0:SQ].rearrange("p (f t) -> p f t", t=SC)
            tT3 = trT[:, :].rearrange("p (f t) -> p f t", t=SC)
            f0b = (feats_sp[:, :].rearrange("p (l j) -> p j l", l=LC)
                   [:, 0:1, 0:SC].broadcast_to([128, SC, SC]))
            v.tensor_tensor(s0, tT3, f0b, op=ADD)
            v.engine_nop()
            for j in range(1, GL):
                step(j)
            v.engine_nop().then_inc(dv_sem, 1)  # group-0 states exportable
            v.wait_ge(sp_sem, 16 * LC * NG)     # group-1 feats in place
            mbuild(1)
            for j in range(GL, CL):
                step(j)
            v.engine_nop().then_inc(dv_sem, 1)

    return nc


_PROG = None


def _get_prog():
    global _PROG
    if _PROG is None:
        _PROG = build_program()
    return _PROG


def _perm_t():
    """u -> t map: u = g*256 + c*16 + jj holds feat time t = 32c+16g+jj+1."""
    u = np.arange(T)
    g = u // GT
    c = (u % GT) // GL
    jj = u % GL
    return 32 * c + GL * g + jj + 1          # in 1..512; 512 -> zero-fill


def make_in_maps(hidden_states, W, b, transitions):
    hs = np.asarray(hidden_states, np.float32)
    W = np.asarray(W, np.float32)
    bb = np.asarray(b, np.float32)
    trans = np.asarray(transitions, np.float32)

    import ml_dtypes
    BF = ml_dtypes.bfloat16
    Wc = W[:, LAB]                                       # [768, 8]
    wk = np.ascontiguousarray(Wc.reshape(KC, 128, LC).transpose(1, 0, 2)
                              ).reshape(128, KC * LC)
    wkh = wk.astype(BF)
    wkl = (wk - wkh.astype(np.float32)).astype(BF)
    trb = trans[0:SC, 0:SC] + bb[0:SC, None]             # [to, f] (+bias)
    trR = np.ascontiguousarray(
        np.broadcast_to(trb.reshape(1, SQ), (128, SQ)))
    trT = np.ascontiguousarray(
        np.broadcast_to(trb.T.reshape(1, SQ), (128, SQ))).copy()
    # chunk-0 rows (r = 16b): seed from START: trans[to,7]+b[to], any f0
    d7 = trans[0:SC, START] + bb[0:SC]
    trT[0::CH, :] = np.broadcast_to(d7[None, :], (SC, SC)).reshape(SQ)

    t_of_u = _perm_t()
    tclip = np.minimum(t_of_u, T - 1)
    dead = t_of_u >= T
    in_maps = []
    for c in range(NC):
        shard = hs[c * BL:(c + 1) * BL]                  # [8, 512, 768]
        hsT = np.ascontiguousarray(shard.transpose(0, 2, 1))  # [8, 768, 512]
        hsP = hsT[:, :, tclip]
        hsP[:, :, dead] = 0.0
        hsTh = hsP.astype(BF)
        hsTl = (hsP - hsTh.astype(np.float32)).astype(BF)
        in_maps.append({"hsTh": hsTh, "hsTl": hsTl, "wkh": wkh, "wkl": wkl,
                        "trR": trR, "trT": trT})
    return in_maps


def kernel(hidden_states, W, b, transitions):
    in_maps = make_in_maps(hidden_states, W, b, transitions)
    nc = _get_prog()
    res = run_bass_kernel_spmd(nc, in_maps, list(range(NC))).results

    bb = np.asarray(b, np.float32)
    trans = np.asarray(transitions, np.float32)
    lab = np.array(LAB, np.int32)

    # scan prefix states: rows r = 16b + c, slots (j, f0, to)
    sall = np.stack([res[c]["sall"] for c in range(NC)])     # [8,128,1568]
    S = sall.reshape(B, CH, CL, SC, SC)                      # [bt,c,j,f0,to]
    feats = np.stack([res[c]["feats"] for c in range(NC)])   # [8,128,256]
    feats = (feats.reshape(NC, BL, CH, LC, CL)
             .transpose(0, 1, 2, 4, 3).reshape(B, T, LC))    # s <-> t=s+1

    # phase 2 (host): boundary deltas across chunks, same fp32 ops as device
    db = np.zeros((B, CH, SC), np.float32)
    for c in range(CH - 1):
        db[:, c + 1] = (S[:, c, CL - 1] + db[:, c][:, :, None]).max(axis=1)
    # phase 3 (host): delta_t = max_f0(S_t[f0,:] + db_c[f0])
    delta = (S + db[:, :, None, :, None]).max(axis=3)        # [bt,c,j,to]
    dall = np.empty((B, T + 1, SC), np.float32)
    dall[:, 1:] = delta.reshape(B, T, SC)                    # slot s <-> t=s+1

    # final step: candidates over compact labels (0..6, STOP)
    d510 = dall[:, T - 2, :]                                 # [64, 7]
    cand = np.empty((B, LC), np.float32)
    cand[:, 0:SC] = dall[:, T - 1, :]
    cand[:, SC] = (np.max(trans[STOP, 0:SC][None, :] + d510, axis=1)
                   + feats[:, T - 2, SC] + bb[STOP])
    p = lab[np.argmax(cand, axis=1)]                         # [64] labels
    path = np.empty((B, T), np.int32)
    path[:, T - 1] = p
    trf = trans[:, 0:SC]                                     # [9, 7]
    for t in range(T - 1, 1, -1):
        p = np.argmax(trf[p] + dall[:, t - 1, :], axis=1).astype(np.int32)
        path[:, t - 1] = p
    path[:, 0] = START
    return path
